# revision 3
# baseline (speedup 1.0000x reference)
"""RNN-T Joiner kernel for Trainium2, data-parallel over (B, T) on 8 cores.

reference:
    logit = tanh(enc[:, :, None, :] + dec[:, None, :, :])   # (B,T,U,C)
    out   = einsum('btuc,vc->btuv', logit, W) + b           # (B,T,U,V)

Shapes (hardcoded): B=4, T=256, U=64, C=512, V=1024.

Sharding: core k handles b = k//2, t rows [ (k%2)*128, (k%2)*128+128 ).
W / bias replicated. No collectives.

Per-core device kernel (C on partitions for the logit):
  - logitT[c, t] = tanh(encT[c, t] + decT[c, u])  -- scalar engine, fused
    per-partition bias add.
  - out[t, v] accumulated over 4 c-chunks of K=128 matmuls; inputs bitcast
    to float32r (full PE rate at out-free-dim >= 256, fp32 data).
  - bias add fused into the PSUM->SBUF eviction on DVE.
  - out tile DMA'd straight to DRAM (2KB contiguous per partition).
"""

import numpy as np


def _ensure_ntff_hook():
    """Make BASS_TRACE work when the image's `antenv` lacks `axon_hooks`.

    bass_utils' axon trace path imports antenv.axon_hooks; some images
    ship antenv without that submodule, so tracing silently degrades.
    Install a minimal module providing the get/set pair, wired to the
    ctypes NTFF hook from trn_agent_boot when available. Purely
    additive: no-op if the real module exists.
    """
    try:
        from antenv.axon_hooks import get_axon_ntff_profile_hook  # noqa: F401

        return
    except ImportError:
        pass
    import sys
    import types

    hook = None
    try:
        from trn_agent_boot.trn_boot import _ntff_profile_via_ctypes

        hook = _ntff_profile_via_ctypes("/opt/axon/libaxon_pjrt.so")
    except Exception:
        hook = None
    mod = types.ModuleType("antenv.axon_hooks")
    mod._hook = hook
    mod.get_axon_ntff_profile_hook = lambda: mod._hook

    def _set(h):
        mod._hook = h

    mod.set_axon_ntff_profile_hook = _set
    sys.modules["antenv.axon_hooks"] = mod
    try:
        import antenv

        antenv.axon_hooks = mod
    except ImportError:
        pass


B, T, U, C, V = 4, 256, 64, 512, 1024
NCORES = 8
TS = 128  # t rows per core
CCH = C // 128  # 4 contraction chunks
VH = V // 512  # 2 psum-width chunks

_CACHE = {}


def _build():
    from contextlib import ExitStack

    import concourse.bacc as bacc
    import concourse.mybir as mybir
    import concourse.tile as tile

    dt = mybir.dt
    f32 = dt.float32
    f32r = dt.float32r

    nc = bacc.Bacc("TRN2", target_bir_lowering=False, debug=False, num_devices=NCORES)
    enc_t = nc.declare_dram_parameter("enc_t", [C, TS], f32, isOutput=False)
    dec_t = nc.declare_dram_parameter("dec_t", [C, U], f32, isOutput=False)
    wt = nc.declare_dram_parameter("wt", [C, V], f32r, isOutput=False)
    bias_rep = nc.declare_dram_parameter("bias_rep", [128, V], f32, isOutput=False)
    out = nc.declare_dram_parameter("out", [TS, U, V], f32, isOutput=True)

    with tile.TileContext(nc) as tc, ExitStack() as ctx:
        const = ctx.enter_context(tc.tile_pool(name="const", bufs=1))
        logit_pool = ctx.enter_context(tc.tile_pool(name="logit", bufs=6))
        psum_pool = ctx.enter_context(tc.tile_pool(name="psum", bufs=4, space="PSUM"))
        out_pool = ctx.enter_context(tc.tile_pool(name="out", bufs=6))

        wt_sb = const.tile([128, CCH * V], f32r, tag="wt")
        enc_sb = const.tile([128, CCH * TS], f32, tag="enc")
        dec_sb = const.tile([128, CCH * U], f32, tag="dec")
        bias_sb = const.tile([128, V], f32, tag="bias")

        nc.sync.dma_start(
            enc_sb[:].rearrange("p (c t) -> p c t", c=CCH),
            enc_t[:].rearrange("(c p) t -> p c t", p=128),
        )
        nc.sync.dma_start(
            dec_sb[:].rearrange("p (c u) -> p c u", c=CCH),
            dec_t[:].rearrange("(c p) u -> p c u", p=128),
        )
        for c in range(CCH):
            nc.sync.dma_start(
                wt_sb[:, c * V : (c + 1) * V], wt[c * 128 : (c + 1) * 128, :]
            )
        nc.sync.dma_start(bias_sb[:], bias_rep[:])

        for u in range(U):
            lg = logit_pool.tile([128, CCH * TS], f32r, tag="lg")
            for c in range(CCH):
                nc.scalar.activation(
                    lg[:, c * TS : (c + 1) * TS],
                    enc_sb[:, c * TS : (c + 1) * TS],
                    mybir.ActivationFunctionType.Tanh,
                    bias=dec_sb[:, c * U + u : c * U + u + 1],
                )
            ps = psum_pool.tile([128, V], f32, tag="ps")
            for vh in range(VH):
                for c in range(CCH):
                    nc.tensor.matmul(
                        ps[:, vh * 512 : (vh + 1) * 512],
                        lhsT=lg[:, c * TS : (c + 1) * TS],
                        rhs=wt_sb[:, c * V + vh * 512 : c * V + vh * 512 + 512],
                        start=(c == 0),
                        stop=(c == CCH - 1),
                    )
            ob = out_pool.tile([128, V], f32, tag="ob")
            nc.vector.tensor_add(ob[:], ps[:], bias_sb[:])
            nc.sync.dma_start(out[:, u, :], ob[:])

    nc.finalize()
    return nc


def _get_nc():
    if "nc" not in _CACHE:
        _CACHE["nc"] = _build()
    return _CACHE["nc"]


def kernel(**inputs):
    enc = np.asarray(inputs["enc_out"], dtype=np.float32)
    dec = np.asarray(inputs["dec_out"], dtype=np.float32)
    W = np.asarray(inputs["W"], dtype=np.float32)
    b = np.asarray(inputs["b"], dtype=np.float32)

    nc = _get_nc()

    wt_np = np.ascontiguousarray(W.T)
    bias_np = np.ascontiguousarray(np.broadcast_to(b, (128, V)))
    in_maps = []
    for k in range(NCORES):
        bb, t0 = k // 2, (k % 2) * TS
        in_maps.append(
            {
                "enc_t": np.ascontiguousarray(enc[bb, t0 : t0 + TS, :].T),
                "dec_t": np.ascontiguousarray(dec[bb].T),
                "wt": wt_np,
                "bias_rep": bias_np,
            }
        )

    _ensure_ntff_hook()
    from concourse.bass_utils import run_bass_kernel_spmd

    res = run_bass_kernel_spmd(nc, in_maps, list(range(NCORES)))
    _CACHE["last_result"] = res

    out = np.empty((B, T, U, V), np.float32)
    for k in range(NCORES):
        bb, t0 = k // 2, (k % 2) * TS
        out[bb, t0 : t0 + TS] = res.results[k]["out"]
    return out



# revision 4
# speedup vs baseline: 1.0329x; 1.0329x over previous
"""RNN-T Joiner kernel for Trainium2, data-parallel over (B, T) on 8 cores.

reference:
    logit = tanh(enc[:, :, None, :] + dec[:, None, :, :])   # (B,T,U,C)
    out   = einsum('btuc,vc->btuv', logit, W) + b           # (B,T,U,V)

Shapes (hardcoded): B=4, T=256, U=64, C=512, V=1024.

Sharding: core k handles b = k//2, t rows [ (k%2)*128, (k%2)*128+128 ).
W / bias replicated. No collectives.

Per-core device kernel (C on partitions for the logit):
  - all inputs host-prepacked to contiguous [128, X] SBUF layouts; input
    DMAs split across sync/scalar/gpsimd queues so descriptor generation
    overlaps.
  - warmup matmuls on a memset junk tile run during the input-DMA
    preamble so the PE HAM clock-gate is released (2.4 GHz) before the
    real matmul stream starts.
  - logitT[c, t] = tanh(encT[c, t] + decT[c, u]) on the scalar engine
    (fused per-partition bias add), output cast to bf16.
  - out[t, v] accumulated over 4 c-chunks of K=128 bf16 matmuls
    (bf16 enables fast weight load; PSUM accumulation stays fp32).
  - bias add fused into the PSUM->SBUF eviction on DVE.
  - out tile DMA'd straight to DRAM (4KB contiguous per partition).
"""

import numpy as np


def _ensure_ntff_hook():
    """Make BASS_TRACE work when the image's `antenv` lacks `axon_hooks`.

    bass_utils' axon trace path imports antenv.axon_hooks; some images
    ship antenv without that submodule, so tracing silently degrades.
    Install a minimal module providing the get/set pair, wired to the
    ctypes NTFF hook from trn_agent_boot when available. Purely
    additive: no-op if the real module exists.
    """
    try:
        from antenv.axon_hooks import get_axon_ntff_profile_hook  # noqa: F401

        return
    except ImportError:
        pass
    import sys
    import types

    hook = None
    try:
        from trn_agent_boot.trn_boot import _ntff_profile_via_ctypes

        hook = _ntff_profile_via_ctypes("/opt/axon/libaxon_pjrt.so")
    except Exception:
        hook = None
    mod = types.ModuleType("antenv.axon_hooks")
    mod._hook = hook
    mod.get_axon_ntff_profile_hook = lambda: mod._hook

    def _set(h):
        mod._hook = h

    mod.set_axon_ntff_profile_hook = _set
    sys.modules["antenv.axon_hooks"] = mod
    try:
        import antenv

        antenv.axon_hooks = mod
    except ImportError:
        pass


B, T, U, C, V = 4, 256, 64, 512, 1024
NCORES = 8
TS = 128  # t rows per core
CCH = C // 128  # 4 contraction chunks
VH = V // 512  # 2 psum-width chunks
NWARM = 8  # HAM warmup matmuls

_CACHE = {}


def _build():
    from contextlib import ExitStack

    import concourse.bacc as bacc
    import concourse.mybir as mybir
    import concourse.tile as tile

    dt = mybir.dt
    f32 = dt.float32
    bf16 = dt.bfloat16

    nc = bacc.Bacc("TRN2", target_bir_lowering=False, debug=False, num_devices=NCORES)
    # all inputs prepacked host-side to the exact SBUF layout
    enc_t = nc.declare_dram_parameter("enc_t", [128, CCH * TS], f32, isOutput=False)
    dec_t = nc.declare_dram_parameter("dec_t", [128, CCH * U], f32, isOutput=False)
    wt = nc.declare_dram_parameter("wt", [128, CCH * V], bf16, isOutput=False)
    bias_rep = nc.declare_dram_parameter("bias_rep", [128, V], f32, isOutput=False)
    out = nc.declare_dram_parameter("out", [TS, U, V], f32, isOutput=True)

    with tile.TileContext(nc) as tc, ExitStack() as ctx:
        const = ctx.enter_context(tc.tile_pool(name="const", bufs=1))
        logit_pool = ctx.enter_context(tc.tile_pool(name="logit", bufs=6))
        psum_pool = ctx.enter_context(tc.tile_pool(name="psum", bufs=4, space="PSUM"))
        out_pool = ctx.enter_context(tc.tile_pool(name="out", bufs=6))

        wt_sb = const.tile([128, CCH * V], bf16, tag="wt")
        enc_sb = const.tile([128, CCH * TS], f32, tag="enc")
        dec_sb = const.tile([128, CCH * U], f32, tag="dec")
        bias_sb = const.tile([128, V], f32, tag="bias")
        wj = const.tile([128, 512], bf16, tag="wj")

        # junk operand for PE warmup; DVE is idle at kernel start
        nc.vector.memset(wj[:], 0.0)
        for i in range(NWARM):
            wps = psum_pool.tile([128, 512], f32, tag="ps")
            nc.tensor.matmul(
                wps[:], lhsT=wj[:, :128], rhs=wj[:], start=True, stop=True
            )

        # input DMAs: contiguous [128, X] -> single descriptor gen each,
        # spread across three DMA-capable queues
        nc.sync.dma_start(enc_sb[:], enc_t[:])
        nc.scalar.dma_start(dec_sb[:], dec_t[:])
        nc.sync.dma_start(wt_sb[:, 0 * V : 1 * V], wt[:, 0 * V : 1 * V])
        nc.sync.dma_start(wt_sb[:, 1 * V : 2 * V], wt[:, 1 * V : 2 * V])
        nc.gpsimd.dma_start(wt_sb[:, 2 * V : 3 * V], wt[:, 2 * V : 3 * V])
        nc.gpsimd.dma_start(wt_sb[:, 3 * V : 4 * V], wt[:, 3 * V : 4 * V])
        nc.gpsimd.dma_start(bias_sb[:], bias_rep[:])

        for u in range(U):
            lg = logit_pool.tile([128, CCH * TS], bf16, tag="lg")
            for c in range(CCH):
                nc.scalar.activation(
                    lg[:, c * TS : (c + 1) * TS],
                    enc_sb[:, c * TS : (c + 1) * TS],
                    mybir.ActivationFunctionType.Tanh,
                    bias=dec_sb[:, c * U + u : c * U + u + 1],
                )
            ps = psum_pool.tile([128, V], f32, tag="ps")
            for c in range(CCH):
                for vh in range(VH):
                    nc.tensor.matmul(
                        ps[:, vh * 512 : (vh + 1) * 512],
                        lhsT=lg[:, c * TS : (c + 1) * TS],
                        rhs=wt_sb[:, c * V + vh * 512 : c * V + vh * 512 + 512],
                        start=(c == 0),
                        stop=(c == CCH - 1),
                    )
            ob = out_pool.tile([128, V], f32, tag="ob")
            nc.vector.tensor_add(ob[:], ps[:], bias_sb[:])
            nc.sync.dma_start(out[:, u, :], ob[:])

    nc.finalize()
    return nc


def _get_nc():
    if "nc" not in _CACHE:
        _CACHE["nc"] = _build()
    return _CACHE["nc"]


def _chunk128(a):
    """[D, X] -> [128, (D//128)*X] with chunk-major free dim."""
    d, x = a.shape
    return np.ascontiguousarray(
        a.reshape(d // 128, 128, x).transpose(1, 0, 2).reshape(128, (d // 128) * x)
    )


def kernel(**inputs):
    import ml_dtypes

    enc = np.asarray(inputs["enc_out"], dtype=np.float32)
    dec = np.asarray(inputs["dec_out"], dtype=np.float32)
    W = np.asarray(inputs["W"], dtype=np.float32)
    b = np.asarray(inputs["b"], dtype=np.float32)

    nc = _get_nc()

    wt_np = _chunk128(np.ascontiguousarray(W.T)).astype(ml_dtypes.bfloat16)
    bias_np = np.ascontiguousarray(np.broadcast_to(b, (128, V)))
    in_maps = []
    for k in range(NCORES):
        bb, t0 = k // 2, (k % 2) * TS
        in_maps.append(
            {
                "enc_t": _chunk128(np.ascontiguousarray(enc[bb, t0 : t0 + TS, :].T)),
                "dec_t": _chunk128(np.ascontiguousarray(dec[bb].T)),
                "wt": wt_np,
                "bias_rep": bias_np,
            }
        )

    _ensure_ntff_hook()
    from concourse.bass_utils import run_bass_kernel_spmd

    res = run_bass_kernel_spmd(nc, in_maps, list(range(NCORES)))
    _CACHE["last_result"] = res

    out = np.empty((B, T, U, V), np.float32)
    for k in range(NCORES):
        bb, t0 = k // 2, (k % 2) * TS
        out[bb, t0 : t0 + TS] = res.results[k]["out"]
    return out


# revision 6
# speedup vs baseline: 1.0421x; 1.0089x over previous
"""RNN-T Joiner kernel for Trainium2, data-parallel over (B, T) on 8 cores.

reference:
    logit = tanh(enc[:, :, None, :] + dec[:, None, :, :])   # (B,T,U,C)
    out   = einsum('btuc,vc->btuv', logit, W) + b           # (B,T,U,V)

Shapes (hardcoded): B=4, T=256, U=64, C=512, V=1024.

Sharding: core k handles b = k//2, t rows [ (k%2)*128, (k%2)*128+128 ).
W / bias replicated. No collectives.

Per-core device kernel (C on partitions for the logit):
  - all inputs host-prepacked to contiguous [128, X] SBUF layouts; input
    DMAs split across sync/scalar/gpsimd queues so descriptor generation
    overlaps.
  - warmup matmuls on a memset junk tile run during the input-DMA
    preamble so the PE HAM clock-gate is released (2.4 GHz) before the
    real matmul stream starts.
  - logitT[c, t] = tanh(encT[c, t] + decT[c, u]) on the scalar engine
    (fused per-partition bias add), output cast to bf16.
  - out[t, v] accumulated over 4 c-chunks of K=128 bf16 matmuls
    (bf16 enables fast weight load; PSUM accumulation stays fp32).
  - bias add fused into the PSUM->SBUF eviction on DVE.
  - out tile DMA'd straight to DRAM (4KB contiguous per partition).
"""

import numpy as np


def _ensure_ntff_hook():
    """Make BASS_TRACE work when the image's `antenv` lacks `axon_hooks`.

    bass_utils' axon trace path imports antenv.axon_hooks; some images
    ship antenv without that submodule, so tracing silently degrades.
    Install a minimal module providing the get/set pair, wired to the
    ctypes NTFF hook from trn_agent_boot when available. Purely
    additive: no-op if the real module exists.
    """
    try:
        from antenv.axon_hooks import get_axon_ntff_profile_hook  # noqa: F401

        return
    except ImportError:
        pass
    import sys
    import types

    hook = None
    try:
        from trn_agent_boot.trn_boot import _ntff_profile_via_ctypes

        hook = _ntff_profile_via_ctypes("/opt/axon/libaxon_pjrt.so")
    except Exception:
        hook = None
    mod = types.ModuleType("antenv.axon_hooks")
    mod._hook = hook
    mod.get_axon_ntff_profile_hook = lambda: mod._hook

    def _set(h):
        mod._hook = h

    mod.set_axon_ntff_profile_hook = _set
    sys.modules["antenv.axon_hooks"] = mod
    try:
        import antenv

        antenv.axon_hooks = mod
    except ImportError:
        pass


B, T, U, C, V = 4, 256, 64, 512, 1024
NCORES = 8
TS = 128  # t rows per core
CCH = C // 128  # 4 contraction chunks
VH = V // 512  # 2 psum-width chunks
NWARM = 8  # HAM warmup matmuls

_CACHE = {}


def _build():
    from contextlib import ExitStack

    import concourse.bacc as bacc
    import concourse.mybir as mybir
    import concourse.tile as tile

    dt = mybir.dt
    f32 = dt.float32
    bf16 = dt.bfloat16

    nc = bacc.Bacc("TRN2", target_bir_lowering=False, debug=False, num_devices=NCORES)
    # all inputs prepacked host-side to the exact SBUF layout
    enc_t = nc.declare_dram_parameter("enc_t", [128, CCH * TS], bf16, isOutput=False)
    dec_t = nc.declare_dram_parameter("dec_t", [128, CCH * U], bf16, isOutput=False)
    wt = nc.declare_dram_parameter("wt", [128, CCH * V], bf16, isOutput=False)
    bias_rep = nc.declare_dram_parameter("bias_rep", [128, V], f32, isOutput=False)
    out = nc.declare_dram_parameter("out", [TS, U, V], f32, isOutput=True)

    with tile.TileContext(nc) as tc, ExitStack() as ctx:
        const = ctx.enter_context(tc.tile_pool(name="const", bufs=1))
        logit_pool = ctx.enter_context(tc.tile_pool(name="logit", bufs=6))
        psum_pool = ctx.enter_context(tc.tile_pool(name="psum", bufs=4, space="PSUM"))
        out_pool = ctx.enter_context(tc.tile_pool(name="out", bufs=8))

        wt_sb = const.tile([128, CCH * V], bf16, tag="wt")
        enc_sb = const.tile([128, CCH * TS], bf16, tag="enc")
        dec_sb = const.tile([128, CCH * U], bf16, tag="dec")
        bias_sb = const.tile([128, V], f32, tag="bias")
        wj = const.tile([128, 512], bf16, tag="wj")

        # junk operand for PE warmup; DVE is idle at kernel start
        nc.vector.memset(wj[:], 0.0)
        for i in range(NWARM):
            wps = psum_pool.tile([128, 512], f32, tag="ps")
            nc.tensor.matmul(
                wps[:], lhsT=wj[:, :128], rhs=wj[:], start=True, stop=True
            )

        # input DMAs: contiguous [128, X] -> single descriptor gen each,
        # spread across three DMA-capable queues with the critical-path
        # pieces (wt c0, enc, dec) first on their rings
        nc.sync.dma_start(wt_sb[:, 0 * V : 1 * V], wt[:, 0 * V : 1 * V])
        nc.scalar.dma_start(enc_sb[:], enc_t[:])
        nc.scalar.dma_start(dec_sb[:], dec_t[:])
        nc.sync.dma_start(wt_sb[:, 1 * V : 2 * V], wt[:, 1 * V : 2 * V])
        nc.gpsimd.dma_start(wt_sb[:, 2 * V : 3 * V], wt[:, 2 * V : 3 * V])
        nc.gpsimd.dma_start(wt_sb[:, 3 * V : 4 * V], wt[:, 3 * V : 4 * V])
        nc.gpsimd.dma_start(bias_sb[:], bias_rep[:])

        for u in range(U):
            lg = logit_pool.tile([128, CCH * TS], bf16, tag="lg")
            for c in range(CCH):
                nc.scalar.activation(
                    lg[:, c * TS : (c + 1) * TS],
                    enc_sb[:, c * TS : (c + 1) * TS],
                    mybir.ActivationFunctionType.Tanh,
                    bias=dec_sb[:, c * U + u : c * U + u + 1],
                )
            ps = psum_pool.tile([128, V], f32, tag="ps")
            for c in range(CCH):
                for vh in range(VH):
                    nc.tensor.matmul(
                        ps[:, vh * 512 : (vh + 1) * 512],
                        lhsT=lg[:, c * TS : (c + 1) * TS],
                        rhs=wt_sb[:, c * V + vh * 512 : c * V + vh * 512 + 512],
                        start=(c == 0),
                        stop=(c == CCH - 1),
                    )
            ob = out_pool.tile([128, V], f32, tag="ob")
            nc.vector.tensor_add(ob[:], ps[:], bias_sb[:])
            nc.sync.dma_start(out[:, u, :], ob[:])

    nc.finalize()
    return nc


def _get_nc():
    if "nc" not in _CACHE:
        _CACHE["nc"] = _build()
    return _CACHE["nc"]


def _chunk128(a):
    """[D, X] -> [128, (D//128)*X] with chunk-major free dim."""
    d, x = a.shape
    return np.ascontiguousarray(
        a.reshape(d // 128, 128, x).transpose(1, 0, 2).reshape(128, (d // 128) * x)
    )


def kernel(**inputs):
    import ml_dtypes

    enc = np.asarray(inputs["enc_out"], dtype=np.float32)
    dec = np.asarray(inputs["dec_out"], dtype=np.float32)
    W = np.asarray(inputs["W"], dtype=np.float32)
    b = np.asarray(inputs["b"], dtype=np.float32)

    nc = _get_nc()

    bf = ml_dtypes.bfloat16
    wt_np = _chunk128(np.ascontiguousarray(W.T)).astype(bf)
    bias_np = np.ascontiguousarray(np.broadcast_to(b, (128, V)))
    in_maps = []
    for k in range(NCORES):
        bb, t0 = k // 2, (k % 2) * TS
        in_maps.append(
            {
                "enc_t": _chunk128(np.ascontiguousarray(enc[bb, t0 : t0 + TS, :].T)).astype(bf),
                "dec_t": _chunk128(np.ascontiguousarray(dec[bb].T)).astype(bf),
                "wt": wt_np,
                "bias_rep": bias_np,
            }
        )

    _ensure_ntff_hook()
    from concourse.bass_utils import run_bass_kernel_spmd

    res = run_bass_kernel_spmd(nc, in_maps, list(range(NCORES)))
    _CACHE["last_result"] = res

    out = np.empty((B, T, U, V), np.float32)
    for k in range(NCORES):
        bb, t0 = k // 2, (k % 2) * TS
        out[bb, t0 : t0 + TS] = res.results[k]["out"]
    return out
